# revision 7
# baseline (speedup 1.0000x reference)
import re
import sys

sys.path.insert(0, "/opt/trn_rl_repo")

import numpy as np

from concourse import bass, mybir
from concourse.bass_utils import run_bass_kernel_spmd
from concourse.tile import TileContext
from concourse.vector_clock import ScopedClock, VectorClock


def _drain_and_barrier_split_waits(self, tick_clock, wait_clock):
    # Walrus codegen caps sync waits per CTRL instruction at 2; the stock
    # exit drain carries one wait per busy proc (3+ with compute + 2 DMA
    # lanes) and fails to compile. Emit one single-wait SP nop per proc
    # ahead of the drain instead; SP executes them in order, so the drain
    # still starts only after all procs are quiesced.
    ticks = [int(v) for v in re.findall(r"\d+", str(tick_clock.global_clock))]
    for p, t in enumerate(ticks):
        if t > 0:
            partial = VectorClock()
            partial.require_at_least(p, t)
            nop = self.nc.sync.nop(nofuse=True)
            wait_clock.add_sem_waits(nop.ins, ScopedClock({None: partial}))
    self.nc.sync.drain()
    self.nc.all_engine_barrier()
    assert self.sems is not None
    popped = self.nc._tile_sem_poison_stack.pop()
    assert popped is self._sem_poison
    self.nc.clear_and_free_semaphores(list(self.sems.allocated().values()))
    self.nc.all_engine_barrier()


TileContext._drain_and_barrier = _drain_and_barrier_split_waits

B, L = 131072, 256
NCORES = 8
RPC = B // NCORES          # rows per core
P = 128                    # partitions
NT = RPC // P              # tiles per core (128)
SENT = 1000.0              # sentinel: never equals/exceeds any id (ids < 40)
DT = mybir.dt.bfloat16
F32 = mybir.dt.float32
Alu = mybir.AluOpType
Act = mybir.ActivationFunctionType

N_LOAD_CHUNKS = 1          # big preload DMAs
TILES_PER_CHUNK = NT // N_LOAD_CHUNKS

_cache = {}


def build():
    nc = bass.Bass()
    x = nc.declare_dram_parameter("x", [RPC, L], mybir.dt.int32, isOutput=False)
    out = nc.declare_dram_parameter("out", [RPC, 4], F32, isOutput=True)

    # tile j, partition p  <->  row p*NT + j
    x_re = x.rearrange("(p n) m -> p (n m)", p=P)        # [128, NT*L]
    out_re = out.rearrange("(p n) k -> p (n k)", p=P)    # [128, NT*4]

    with TileContext(nc) as tc:
        with (
            tc.tile_pool(name="persist", bufs=1) as pp,
            tc.tile_pool(name="work", bufs=4) as wp,
        ):
            xall = pp.tile([P, NT * L], DT)
            rep_s = pp.tile([P, NT], F32)
            ge_s = pp.tile([P, NT], F32)
            per_s = pp.tile([P, NT], F32)
            zc_s = pp.tile([P, NT], F32)
            feat = pp.tile([P, NT * 4], F32)

            # preload whole shard, int32 -> bf16 cast in SWDGE
            CW = TILES_PER_CHUNK * L
            for c in range(N_LOAD_CHUNKS):
                nc.gpsimd.dma_start(
                    out=xall[:, c * CW:(c + 1) * CW],
                    in_=x_re[:, c * CW:(c + 1) * CW],
                )

            for j in range(NT):
                xb = xall[:, j * L:(j + 1) * L]

                # m = 1.0 where x==0 else 0.0 ; accum = zero count per row
                m = wp.tile([P, L], DT, tag="m")
                nc.vector.scalar_tensor_tensor(
                    out=m, in0=xb, scalar=0.5, in1=xb,
                    op0=Alu.is_lt, op1=Alu.bypass,
                    accum_out=zc_s[:, j:j + 1],
                )

                # mz = 1.0 where x!=0 else 0.0
                mz = wp.tile([P, L], DT, tag="mz")
                nc.vector.tensor_scalar(mz, xb, 0.5, None, Alu.is_gt)

                # v1p[:, t] = last nonzero strictly before t (SENT if none)
                v1p = wp.tile([P, L + 2], DT, tag="v1p")
                nc.vector.memset(v1p[:, 0:1], SENT)
                nc.vector.tensor_tensor_scan(
                    out=v1p[:, 1:L + 1], data0=m, data1=xb,
                    initial=SENT, op0=Alu.mult, op1=Alu.add,
                )
                v1 = v1p[:, 0:L]

                # b = v1p where x!=0 else 0  (scan-2 injection values)
                b = wp.tile([P, L], DT, tag="b")
                nc.vector.tensor_tensor(b, mz, v1, Alu.mult)

                # v2p[:, t] = second-previous nonzero strictly before t
                v2p = wp.tile([P, L + 2], DT, tag="v2p")
                nc.vector.memset(v2p[:, 0:1], SENT)
                nc.vector.tensor_tensor_scan(
                    out=v2p[:, 1:L + 1], data0=m, data1=b,
                    initial=SENT, op0=Alu.mult, op1=Alu.add,
                )
                v2 = v2p[:, 0:L]

                # fused compare + row-sum reduces
                s0 = wp.tile([P, L], DT, tag="s0")
                nc.vector.scalar_tensor_tensor(
                    out=s0, in0=xb, scalar=1.0, in1=v1,
                    op0=Alu.mult, op1=Alu.is_equal,
                    accum_out=rep_s[:, j:j + 1],
                )
                s1 = wp.tile([P, L], DT, tag="s1")
                nc.vector.scalar_tensor_tensor(
                    out=s1, in0=xb, scalar=1.0, in1=v1,
                    op0=Alu.mult, op1=Alu.is_ge,
                    accum_out=ge_s[:, j:j + 1],
                )
                s2 = wp.tile([P, L], DT, tag="s2")
                nc.vector.scalar_tensor_tensor(
                    out=s2, in0=xb, scalar=1.0, in1=v2,
                    op0=Alu.mult, op1=Alu.is_equal,
                    accum_out=per_s[:, j:j + 1],
                )

            # epilogue: [P, NT] fp32 stat tensors -> interleaved feat tile
            ep = pp
            n = ep.tile([P, NT], F32)
            nc.vector.tensor_scalar(n, zc_s, -1.0, float(L), Alu.mult, Alu.add)
            nm1 = ep.tile([P, NT], F32)
            nc.vector.tensor_scalar(nm1, n, -1.0, None, Alu.add)
            d1 = ep.tile([P, NT], F32)
            nc.vector.tensor_scalar(d1, nm1, 1.0, None, Alu.max)
            r1 = ep.tile([P, NT], F32)
            nc.vector.reciprocal(r1, d1)
            d2 = ep.tile([P, NT], F32)
            nc.vector.tensor_scalar(d2, n, -2.0, 1.0, Alu.add, Alu.max)
            r2 = ep.tile([P, NT], F32)
            nc.vector.reciprocal(r2, d2)

            mask2 = ep.tile([P, NT], F32)
            nc.vector.tensor_scalar(mask2, n, 1.5, None, Alu.is_gt)
            mask4 = ep.tile([P, NT], F32)
            nc.vector.tensor_scalar(mask4, n, 3.5, None, Alu.is_gt)

            r1m = ep.tile([P, NT], F32)
            nc.vector.tensor_tensor(r1m, r1, mask2, Alu.mult)

            feat3 = feat.rearrange("p (n k) -> p n k", k=4)
            # rep_ratio
            nc.vector.tensor_tensor(feat3[:, :, 0:1], rep_s, r1m, Alu.mult)
            # inc = ge - rep ; inc_ratio
            inc = ep.tile([P, NT], F32)
            nc.vector.tensor_tensor(inc, ge_s, rep_s, Alu.subtract)
            nc.vector.tensor_tensor(feat3[:, :, 1:2], inc, r1m, Alu.mult)
            # dec = nm1 - ge ; dec_ratio
            dec = ep.tile([P, NT], F32)
            nc.vector.tensor_tensor(dec, nm1, ge_s, Alu.subtract)
            nc.vector.tensor_tensor(feat3[:, :, 2:3], dec, r1m, Alu.mult)
            # periodicity
            r2m = ep.tile([P, NT], F32)
            nc.vector.tensor_tensor(r2m, r2, mask4, Alu.mult)
            nc.vector.tensor_tensor(feat3[:, :, 3:4], per_s, r2m, Alu.mult)

            nc.gpsimd.dma_start(out=out_re, in_=feat)

    return nc


def kernel(x: np.ndarray) -> np.ndarray:
    if "nc" not in _cache:
        _cache["nc"] = build()
    nc = _cache["nc"]
    x = np.ascontiguousarray(np.asarray(x, dtype=np.int32))
    shards = x.reshape(NCORES, RPC, L)
    in_maps = [{"x": shards[i]} for i in range(NCORES)]
    res = run_bass_kernel_spmd(nc, in_maps, list(range(NCORES)))
    outs = [res.results[i]["out"] for i in range(NCORES)]
    return np.concatenate(outs, axis=0).astype(np.float32)
